# revision 36
# baseline (speedup 1.0000x reference)
"""Trainium2 Bass kernel for the contrastive loss (nn_Contrast).

loss = LAM * mean_i(-log s_mp[i]) + (1-LAM) * mean_i(-log s_sc[i])
  S = exp(cos(n1_i, n2_j)/tau);  n1 = norm(proj(z_mp)), n2 = norm(proj(z_sc))
  s_mp[i] = sum_d S[i, c_id] / rowsum_i ;  s_sc[i] = sum_d S[c_id, i] / colsum_i

Sharding: rows of S across 8 cores (1024 rows each). Per core:
  - project its z_mp row block (bf16 transposed pipeline), fold 1/(norm*tau)
    into the exp scale
  - project + normalize its 1024-row slice of z_sc, AllGather the normalized
    n2 across cores (1MB/core) so no core needs the full z_sc from the host
  - stream its [1024, 8192] S block tile-by-tile: exp with row scale
    (rowsum via ACT accum), colsum via f32 ones-matmul, and the sparse
    numerators via on-device masks built from compact edge-index tensors:
      mp: per edge slot d, (col_iota == c_d[row]) * S accumulated row-wise
      sc: mask[p, i] = sum_d (c_d_bcast[i] == rowidx[p]) then masked colsum
  - outputs are only the reductions (~360KB/core); the host sums slots and
    takes the final logs/means.
"""

import numpy as np
import ml_dtypes

N = 8192
HID = 512
TAU = 0.8
LAM = 0.5
NCORES = 8
B = N // NCORES          # rows per core = 1024
RT = B // 128            # row tiles per core = 8
CC = N // 1024           # 1024-wide col chunks = 8
KT = HID // 128          # contraction tiles = 4
DEG = 8                  # padded positives per node

bf16 = ml_dtypes.bfloat16
fp8 = ml_dtypes.float8_e4m3


def _split_multi_waits(nc, mybir):
    """This container's walrus accepts only ONE sync-wait per instruction;
    Tile batches several. Split extras into single-wait NoOps."""
    counter = [0]
    for f in nc.m.functions:
        for bb in f.blocks:
            new_insts = []
            changed = False
            for inst in bb.instructions:
                si = inst.sync_info
                if si is not None and si.on_wait is not None and len(si.on_wait) > 1:
                    waits = list(si.on_wait)
                    for w in waits[:-1]:
                        counter[0] += 1
                        new_insts.append(mybir.InstNoOp(
                            name=f"I-wsplit-{counter[0]}",
                            engine=inst.engine,
                            sync_info=mybir.SyncInfo(on_wait=[w], on_update=[]),
                            bass_nofuse=True,
                        ))
                    inst.sync_info = mybir.SyncInfo(
                        on_wait=[waits[-1]], on_update=list(si.on_update or []))
                    changed = True
                new_insts.append(inst)
            if changed:
                bb.instructions = new_insts
    return nc


def build_program(deg: int = DEG):
    import concourse.bass as bass
    import concourse.mybir as mybir
    import concourse.tile as tile

    dt = mybir.dt
    F32, BF16 = dt.float32, dt.bfloat16
    Act = mybir.ActivationFunctionType
    Alu = mybir.AluOpType

    F8 = dt.float8e4
    F16 = dt.float16

    nc = bass.Bass("TRN2", num_devices=NCORES)

    # z_mp block (rows 0:HID) stacked over z_sc block (rows HID:2*HID)
    zt = nc.dram_tensor("zt", [2 * HID, B], F8, kind="ExternalInput")
    # this core's 128-row slice of [W1.T ; W2.T] — all-gathered on device
    ws_sh = nc.dram_tensor("ws_sh", [2 * HID // NCORES, HID], BF16,
                           kind="ExternalInput")
    b1r = nc.dram_tensor("b1r", [1, HID], BF16, kind="ExternalInput")
    b2r = nc.dram_tensor("b2r", [1, HID], BF16, kind="ExternalInput")
    # mp edges: global col index of the d-th positive of each local row
    cmp_d = nc.dram_tensor("cmp_d", [RT, 128, deg], F32, kind="ExternalInput")
    # sc edges: local row (c - k*B, or -1 if off-core) of the d-th positive
    # of each global row (=S column); values in [-1, 1023] are fp16-exact
    csc_d = nc.dram_tensor("csc_d", [CC, deg, 1024], F16, kind="ExternalInput")
    rowidx = nc.dram_tensor("rowidx", [128, RT], F32, kind="ExternalInput")
    iota_r = nc.dram_tensor("iota_r", [1, 1024], F32, kind="ExternalInput")

    NSLOT = RT * CC
    out_rows = nc.dram_tensor("out_rows", [128, 2 * RT], F32,
                              kind="ExternalOutput")
    out_cols = nc.dram_tensor("out_cols", [2, N], BF16, kind="ExternalOutput")

    rn1_dram = nc.dram_tensor("rn1_dram", [B], F32)
    ag_in = nc.dram_tensor("ag_in", [HID, B], BF16)
    ag_out = nc.dram_tensor("ag_out", [NCORES * HID, B], BF16,
                            addr_space="Shared")
    agw_out = nc.dram_tensor("agw_out", [2 * HID, HID], BF16,
                             addr_space="Shared")
    ws_stage = nc.dram_tensor("ws_stage", [2 * HID // NCORES, HID], BF16)

    with tile.TileContext(nc) as tc:
        with tc.tile_pool(name="const", bufs=1) as constp, \
             tc.tile_pool(name="persist", bufs=1) as pers:
            ones_row = constp.tile([1, 1024], BF16, tag="ones_row", name="ones_row")
            nc.vector.memset(ones_row[:], 1.0)
            ones_row_f32 = constp.tile([1, 128], F32, tag="ones_row_f32", name="ones_row_f32")
            nc.vector.memset(ones_row_f32[:], 1.0)
            ones_row_f16 = constp.tile([1, 128], F16, tag="ones_row_f16", name="ones_row_f16")
            nc.vector.memset(ones_row_f16[:], 1.0)
            ones_col = constp.tile([128, 1], BF16, tag="ones_col", name="ones_col")
            nc.vector.memset(ones_col[:], 1.0)
            ones_col_f32 = constp.tile([128, 1], F32, tag="ones_col_f32", name="ones_col_f32")
            nc.vector.memset(ones_col_f32[:], 1.0)

            # weights arrive as a 128-row slice per core; gather on device
            # (collectives cannot read IO tensors -> stage through scratch)
            nc.sync.dma_start(out=ws_stage[:], in_=ws_sh[:])
            nc.gpsimd.collective_compute(
                "AllGather", mybir.AluOpType.bypass,
                replica_groups=[list(range(NCORES))],
                ins=[ws_stage[:]], outs=[agw_out[:]])
            w1s = [constp.tile([128, HID], BF16, tag=f"w1_{k}", name=f"w1_{k}") for k in range(KT)]
            w2s = [constp.tile([128, HID], BF16, tag=f"w2_{k}", name=f"w2_{k}") for k in range(KT)]
            for k in range(KT):
                nc.sync.dma_start(out=w1s[k][:], in_=agw_out[k * 128:(k + 1) * 128, :])
                nc.sync.dma_start(out=w2s[k][:],
                                  in_=agw_out[HID + k * 128:HID + (k + 1) * 128, :])
            b1s = constp.tile([1, HID], BF16, tag="b1s", name="b1s")
            nc.sync.dma_start(out=b1s[:], in_=b1r[:])
            b2s = constp.tile([1, HID], BF16, tag="b2s", name="b2s")
            nc.sync.dma_start(out=b2s[:], in_=b2r[:])

            cmp_sb = constp.tile([128, RT * deg], F32, tag="cmp_sb", name="cmp_sb")
            for rt in range(RT):
                nc.sync.dma_start(out=cmp_sb[:, rt * deg:(rt + 1) * deg],
                                  in_=cmp_d[rt])
            rowidx_sb = constp.tile([128, RT], F32, tag="rowidx_sb", name="rowidx_sb")
            nc.sync.dma_start(out=rowidx_sb[:], in_=rowidx[:])
            iota_sb = constp.tile([1, 1024], F32, tag="iota_sb", name="iota_sb")
            nc.sync.dma_start(out=iota_sb[:], in_=iota_r[:])

            # persistent results
            p1T = [pers.tile([128, B], BF16, tag=f"p1T_{k}", name=f"p1T_{k}") for k in range(KT)]
            n2T = [pers.tile([128, N], BF16, tag=f"n2T_{k}", name=f"n2T_{k}") for k in range(KT)]
            scale_mp = pers.tile([128, RT], F32, tag="scale_mp", name="scale_mp")
            rowsum_acc = pers.tile([128, NSLOT], F32, tag="rowsum_acc", name="rowsum_acc")
            nummp_acc = pers.tile([128, NSLOT * deg], F32, tag="nummp_acc", name="nummp_acc")
            iota_bc = pers.tile([128, 1024], F32, tag="iota_bc", name="iota_bc")

            # broadcast iota row down all 128 partitions via PE
            with tc.tile_pool(name="psI", bufs=1, space="PSUM") as psI:
                psio = psI.tile([128, 1024], F32, tag="psio", name="psio")
                for h in range(2):
                    sl = slice(h * 512, (h + 1) * 512)
                    nc.tensor.matmul(psio[:, sl], ones_row_f32[:],
                                     iota_sb[0:1, sl], start=True, stop=True)
                nc.scalar.copy(iota_bc[:], psio[:])

            # ---------------- Stage A/B: project z_mp block and z_sc slice
            # identical transposed pipeline; z_sc output gets normalized and
            # all-gathered, z_mp keeps 1/(norm*tau) as the future exp scale.
            for stage in range(2):
                zoff = stage * HID
                with tc.tile_pool(name=f"st{stage}", bufs=1) as stp, \
                     tc.tile_pool(name=f"wk{stage}", bufs=2) as wkp, \
                     tc.tile_pool(name=f"ps{stage}", bufs=2, space="PSUM") as psp, \
                     tc.tile_pool(name=f"ps{stage}n", bufs=1, space="PSUM") as pspn:
                    zin = [stp.tile([128, B], F8, tag=f"zin_{k}", name=f"zin{stage}_{k}")
                           for k in range(KT)]
                    for k in range(KT):
                        nc.sync.dma_start(
                            out=zin[k][:],
                            in_=zt[zoff + k * 128:zoff + (k + 1) * 128, :])
                    h1 = [stp.tile([128, B], BF16, tag=f"h1_{k}", name=f"h1{stage}_{k}")
                          for k in range(KT)]
                    for ht in range(KT):
                        hsl = slice(ht * 128, (ht + 1) * 128)
                        ps = psp.tile([128, B], F32, tag="ps", name=f"ps{stage}_{ht}")
                        for h in range(B // 512):
                            sl = slice(h * 512, (h + 1) * 512)
                            for k in range(KT):
                                nc.tensor.matmul(ps[:, sl], w1s[k][:, hsl],
                                                 zin[k][:, sl],
                                                 start=(k == 0), stop=False)
                            nc.tensor.matmul(ps[:, sl], b1s[0:1, hsl],
                                             ones_row[0:1, 0:512],
                                             start=False, stop=True)
                        tmin = wkp.tile([128, B], BF16, tag="tmin", name=f"tmin{stage}")
                        nc.vector.tensor_scalar_min(tmin[:], ps[:], 0.0)
                        texp = wkp.tile([128, B], BF16, tag="texp", name=f"texp{stage}")
                        nc.scalar.activation(texp[:], tmin[:], Act.Exp)
                        nc.vector.scalar_tensor_tensor(h1[ht][:], texp[:], -1.0, ps[:],
                                                       op0=Alu.add, op1=Alu.max)
                    norm2h = [pspn.tile([1, 512], F32, tag=f"norm2_{h}",
                                        name=f"norm2{stage}_{h}")
                              for h in range(B // 512)]
                    pT = p1T if stage == 0 else \
                        [stp.tile([128, B], BF16, tag=f"p2T_{k}", name=f"p2T_{k}")
                         for k in range(KT)]
                    for ht in range(KT):
                        hsl = slice(ht * 128, (ht + 1) * 128)
                        ps2 = psp.tile([128, B], F32, tag="ps", name=f"ps2{stage}_{ht}")
                        for h in range(B // 512):
                            sl = slice(h * 512, (h + 1) * 512)
                            for k in range(KT):
                                nc.tensor.matmul(ps2[:, sl], w2s[k][:, hsl],
                                                 h1[k][:, sl],
                                                 start=(k == 0), stop=False)
                            nc.tensor.matmul(ps2[:, sl], b2s[0:1, hsl],
                                             ones_row[0:1, 0:512],
                                             start=False, stop=True)
                        sq = wkp.tile([128, B], BF16, tag="sq", name=f"sq{stage}")
                        nc.scalar.activation(sq[:], ps2[:], Act.Square)
                        for h in range(B // 512):
                            sl = slice(h * 512, (h + 1) * 512)
                            nc.tensor.matmul(norm2h[h][0:1, :], ones_col[:], sq[:, sl],
                                             start=(ht == 0), stop=(ht == KT - 1))
                        nc.vector.tensor_copy(pT[ht][:], ps2[:])
                    nrm = wkp.tile([1, B], F32, tag="nrm", name=f"nrm{stage}")
                    for h in range(B // 512):
                        sl = slice(h * 512, (h + 1) * 512)
                        nc.scalar.activation(nrm[0:1, sl], norm2h[h][:], Act.Sqrt)
                    rn1 = wkp.tile([1, B], F32, tag="rn1", name=f"rn1{stage}")
                    nc.vector.reciprocal(rn1[:], nrm[:])
                    if stage == 0:
                        # scale_mp[p, rt] = 1/(norm*tau) for row rt*128+p
                        nc.vector.tensor_scalar_mul(rn1[:], rn1[:], 1.0 / TAU)
                        nc.gpsimd.dma_start(out=rn1_dram[:], in_=rn1[:])
                        nc.gpsimd.dma_start(
                            out=scale_mp[:],
                            in_=rn1_dram[:].rearrange("(g p) -> p g", p=128))
                    else:
                        # normalize columns of p2T -> own n2 slice, to DRAM
                        rbc = pspn.tile([128, B], F32, tag="rbc", name="rbc")
                        for h in range(B // 512):
                            sl = slice(h * 512, (h + 1) * 512)
                            nc.tensor.matmul(rbc[:, sl], ones_row_f32[:],
                                             rn1[0:1, sl], start=True, stop=True)
                        rbc_sb = wkp.tile([128, B], BF16, tag="rbc_sb", name="rbc_sb")
                        nc.scalar.copy(rbc_sb[:], rbc[:])
                        n2sl = wkp.tile([128, B], BF16, tag="n2sl", name="n2sl")
                        for kk in range(KT):
                            nc.vector.tensor_tensor(n2sl[:], pT[kk][:], rbc_sb[:],
                                                    op=Alu.mult)
                            nc.sync.dma_start(
                                out=ag_in[kk * 128:(kk + 1) * 128, :],
                                in_=n2sl[:])

            # ---------------- AllGather normalized n2 slices -> full n2T
            nc.gpsimd.collective_compute(
                "AllGather", mybir.AluOpType.bypass,
                replica_groups=[list(range(NCORES))],
                ins=[ag_in[:]], outs=[ag_out[:]])
            for k2 in range(NCORES):
                for kk in range(KT):
                    nc.sync.dma_start(
                        out=n2T[kk][:, k2 * B:(k2 + 1) * B],
                        in_=ag_out[k2 * HID + kk * 128:k2 * HID + (kk + 1) * 128, :])

            # ---------------- Stage C: S block sweep
            with tc.tile_pool(name="workC", bufs=2) as wkC, \
                 tc.tile_pool(name="cbcp", bufs=1) as cbcp, \
                 tc.tile_pool(name="cscp", bufs=2) as cscp, \
                 tc.tile_pool(name="psC", bufs=1, space="PSUM") as psC, \
                 tc.tile_pool(name="psCb", bufs=1, space="PSUM") as psCb, \
                 tc.tile_pool(name="psCa", bufs=1, space="PSUM") as psCa:
                for cc in range(CC):
                    # broadcast this chunk's sc edge indices to all partitions
                    cbc = []
                    for d in range(deg):
                        csc_t = cscp.tile([1, 1024], F16, tag="csc_t",
                                          name=f"csc_{cc}_{d}")
                        nc.sync.dma_start(out=csc_t[:], in_=csc_d[cc, d:d + 1, :])
                        pb = psCb.tile([128, 1024], F32, tag="pb", name=f"pb_{cc}_{d}")
                        for h in range(2):
                            sl = slice(h * 512, (h + 1) * 512)
                            nc.tensor.matmul(pb[:, sl], ones_row_f16[:],
                                             csc_t[0:1, sl],
                                             start=True, stop=True)
                        ct = cbcp.tile([128, 1024], F32, tag=f"cbc_{d}",
                                       name=f"cbc_{d}")
                        nc.scalar.copy(ct[:], pb[:])
                        cbc.append(ct)
                    iota_cc = cbcp.tile([128, 1024], F32, tag="iota_cc",
                                        name="iota_cc")
                    nc.vector.tensor_scalar_add(iota_cc[:], iota_bc[:],
                                                float(cc * 1024))

                    csum = [psCa.tile([1, 512], F32, tag=f"csum_{h}", name=f"csum_{h}")
                            for h in range(2)]
                    nsum = [psCa.tile([1, 512], F32, tag=f"nsum_{h}", name=f"nsum_{h}")
                            for h in range(2)]
                    for rt in range(RT):
                        rsl = slice(rt * 128, (rt + 1) * 128)
                        sp = psC.tile([128, 1024], F32, tag="spC", name="spC")
                        for k in range(KT):
                            for h in range(2):
                                sl = slice(cc * 1024 + h * 512,
                                           cc * 1024 + (h + 1) * 512)
                                psl = slice(h * 512, (h + 1) * 512)
                                nc.tensor.matmul(sp[:, psl], p1T[k][:, rsl],
                                                 n2T[k][:, sl],
                                                 start=(k == 0),
                                                 stop=(k == KT - 1))
                        idx = rt * CC + cc
                        s_f32 = wkC.tile([128, 1024], F32, tag="s_f32", name="s_f32")
                        nc.scalar.activation(s_f32[:], sp[:], Act.Exp,
                                             scale=scale_mp[:, rt:rt + 1],
                                             accum_out=rowsum_acc[:, idx:idx + 1])
                        # mp numerators: (iota == c_d) * S, row-accumulated
                        scr = wkC.tile([128, 1024], F32, tag="scr", name="scr")
                        for d in range(deg):
                            so = idx * deg + d
                            nc.vector.scalar_tensor_tensor(
                                scr[:], iota_cc[:],
                                cmp_sb[:, rt * deg + d:rt * deg + d + 1],
                                s_f32[:], op0=Alu.is_equal, op1=Alu.mult,
                                accum_out=nummp_acc[:, so:so + 1])
                        # sc mask: sum_d (cbc_d == rowidx), ping-pong chain
                        mska = wkC.tile([128, 1024], F32, tag="mska", name="mska")
                        mskb = wkC.tile([128, 1024], F32, tag="mskb", name="mskb")
                        nc.vector.tensor_scalar(mska[:], cbc[0][:],
                                                rowidx_sb[:, rt:rt + 1], None,
                                                op0=Alu.is_equal)
                        cur, nxt = mska, mskb
                        for d in range(1, deg):
                            nc.vector.scalar_tensor_tensor(
                                nxt[:], cbc[d][:],
                                rowidx_sb[:, rt:rt + 1], cur[:],
                                op0=Alu.is_equal, op1=Alu.add)
                            cur, nxt = nxt, cur
                        msk = wkC.tile([128, 1024], F32, tag="msk", name="msk")
                        nc.vector.tensor_tensor(msk[:], s_f32[:], cur[:],
                                                op=Alu.mult)
                        for h in range(2):
                            psl = slice(h * 512, (h + 1) * 512)
                            nc.tensor.matmul(csum[h][0:1, :], ones_col_f32[:],
                                             s_f32[:, psl],
                                             start=(rt == 0), stop=(rt == RT - 1))
                            nc.tensor.matmul(nsum[h][0:1, :], ones_col_f32[:],
                                             msk[:, psl],
                                             start=(rt == 0), stop=(rt == RT - 1))
                    for h in range(2):
                        lo = cc * 1024 + h * 512
                        cb = wkC.tile([1, 512], BF16, tag="cb", name="cb")
                        nc.scalar.copy(cb[:], csum[h][:])
                        nc.sync.dma_start(out=out_cols[0, lo:lo + 512], in_=cb[:])
                        nb = wkC.tile([1, 512], BF16, tag="nb", name="nb")
                        nc.scalar.copy(nb[:], nsum[h][:])
                        nc.sync.dma_start(out=out_cols[1, lo:lo + 512], in_=nb[:])

                # reduce slot accumulators to per-row totals before DMA out
                red = wkC.tile([128, 2 * RT], F32, tag="red", name="red")
                for rt in range(RT):
                    nc.vector.reduce_sum(
                        red[:, rt:rt + 1],
                        rowsum_acc[:, rt * CC:(rt + 1) * CC],
                        axis=mybir.AxisListType.X)
                    nc.vector.reduce_sum(
                        red[:, RT + rt:RT + rt + 1],
                        nummp_acc[:, rt * CC * deg:(rt + 1) * CC * deg],
                        axis=mybir.AxisListType.X)
                nc.sync.dma_start(out=out_rows[:], in_=red[:])

    _split_multi_waits(nc, mybir)
    return nc


def _group_cols_by_row(r, c, deg_min=DEG):
    """cols_by_row[i, d] = col of the d-th edge with row i, padded with -1."""
    E = r.shape[0]
    counts = np.bincount(r, minlength=N)
    deg = max(int(counts.max()), deg_min)
    order = np.argsort(r, kind="stable")
    rr = r[order]
    cc = c[order]
    starts = np.cumsum(counts) - counts
    slot = np.arange(E, dtype=np.int64) - starts[rr]
    cols_by_row = np.full((N, deg), -1.0, dtype=np.float32)
    cols_by_row[rr, slot] = cc.astype(np.float32)
    return cols_by_row, deg


def make_in_maps(z_mp, z_sc, W1, b1, W2, b2, pos):
    z_mp = np.asarray(z_mp, dtype=np.float32)
    z_sc = np.asarray(z_sc, dtype=np.float32)
    W1 = np.asarray(W1, dtype=np.float32)
    W2 = np.asarray(W2, dtype=np.float32)
    b1 = np.asarray(b1, dtype=np.float32)
    b2 = np.asarray(b2, dtype=np.float32)
    r = np.asarray(pos[0]).astype(np.int64)
    c = np.asarray(pos[1]).astype(np.int64)

    ws = np.vstack([W1.T, W2.T]).astype(bf16)  # [2*HID, HID]
    b1r = b1.reshape(1, HID).astype(bf16)
    b2r = b2.reshape(1, HID).astype(bf16)

    cols_by_row, deg = _group_cols_by_row(r, c)
    # [CC, deg, 1024] view of cols_by_row for the sc side
    csc_all = np.ascontiguousarray(
        cols_by_row.reshape(CC, 1024, deg).transpose(0, 2, 1))
    rowidx = (np.arange(RT, dtype=np.float32)[None, :] * 128
              + np.arange(128, dtype=np.float32)[:, None]).copy()
    iota_r = np.arange(1024, dtype=np.float32).reshape(1, 1024)
    wrows = 2 * HID // NCORES

    in_maps = []
    for k in range(NCORES):
        rows = slice(k * B, (k + 1) * B)
        zt = np.vstack([z_mp[rows].T, z_sc[rows].T]).astype(fp8)
        cmp_d = np.ascontiguousarray(
            cols_by_row[rows].reshape(RT, 128, deg))
        loc = csc_all - np.float32(k * B)
        csc_d = np.where((loc >= 0) & (loc < B), loc, -1.0).astype(np.float16)
        in_maps.append({
            "zt": zt, "ws_sh": ws[k * wrows:(k + 1) * wrows],
            "b1r": b1r, "b2r": b2r,
            "cmp_d": cmp_d, "csc_d": csc_d,
            "rowidx": rowidx, "iota_r": iota_r,
        })
    return in_maps, deg


def combine_outputs(results, deg=DEG):
    NSLOT = RT * CC
    rowsum = np.empty(N, dtype=np.float64)
    nummp = np.empty(N, dtype=np.float64)
    for k, res in enumerate(results):
        a = np.asarray(res["out_rows"], dtype=np.float64)
        rowsum[k * B:(k + 1) * B] = a[:, :RT].T.reshape(B)
        nummp[k * B:(k + 1) * B] = a[:, RT:].T.reshape(B)
    cols = np.zeros((2, N), dtype=np.float64)
    for res in results:
        cols += np.asarray(res["out_cols"], dtype=np.float64)
    colsum, numsc = cols[0], cols[1]
    term_mp = -np.log(nummp / rowsum).mean()
    term_sc = -np.log(numsc / colsum).mean()
    return np.float32(LAM * term_mp + (1.0 - LAM) * term_sc)


# tensors identical across cores — sent replicated instead of 8x concatenated
_REPLICATED = {"b1r", "b2r", "rowidx", "iota_r"}

_RUNNER_CACHE = {}


def _make_runner(deg):
    """Build the bass program once and wrap it in a cached jitted shard_map
    callable (the equivalent of bass2jax.run_bass_via_pjrt, minus the
    per-call retrace/recompile)."""
    import jax
    import jax.numpy as jnp
    from jax.sharding import Mesh, PartitionSpec
    from jax.experimental.shard_map import shard_map
    from concourse import bass2jax, mybir

    bass2jax.install_neuronx_cc_hook()
    nc = build_program(deg)
    assert not nc.dbg_callbacks
    # dbg_addr is an unused ExternalInput when no callbacks registered;
    # bind zeros so the NEFF tensor is satisfied (uint32[1,2], see bass2jax)
    dbg_name = nc.dbg_addr.name if nc.dbg_addr is not None else None
    dbg_zero = np.zeros((1, 2), np.uint32)

    partition_name = nc.partition_id_tensor.name if nc.partition_id_tensor else None
    in_names, out_names, out_avals, zero_outs = [], [], [], []
    for alloc in nc.m.functions[0].allocations:
        if not isinstance(alloc, mybir.MemoryLocationSet):
            continue
        name = alloc.memorylocations[0].name
        if alloc.kind == "ExternalInput":
            if name != partition_name:
                in_names.append(name)
        elif alloc.kind == "ExternalOutput":
            shape = tuple(alloc.tensor_shape)
            dtype = mybir.dt.np(alloc.dtype)
            out_names.append(name)
            out_avals.append(jax.core.ShapedArray(shape, dtype))
            zero_outs.append(np.zeros(shape, dtype))
    n_params = len(in_names)
    n_outs = len(out_avals)
    all_in_names = in_names + out_names + ([partition_name] if partition_name else [])
    donate = tuple(range(n_params, n_params + n_outs))

    def _body(*args):
        operands = list(args)
        if partition_name is not None:
            operands.append(bass2jax.partition_id_tensor())
        outs = bass2jax._bass_exec_p.bind(
            *operands,
            out_avals=tuple(out_avals),
            in_names=tuple(all_in_names),
            out_names=tuple(out_names),
            lowering_input_output_aliases=(),
            sim_require_finite=True,
            sim_require_nnan=True,
            nc=nc,
        )
        return tuple(outs)

    devices = jax.devices()[:NCORES]
    mesh = Mesh(np.asarray(devices), ("core",))
    repl = _REPLICATED | ({dbg_name} if dbg_name else set())
    in_specs = tuple(
        PartitionSpec() if name in repl else PartitionSpec("core")
        for name in in_names
    ) + (PartitionSpec("core"),) * n_outs
    out_specs = (PartitionSpec("core"),) * n_outs
    sharded = jax.jit(
        shard_map(_body, mesh=mesh, in_specs=in_specs, out_specs=out_specs,
                  check_rep=False),
        donate_argnums=donate, keep_unused=True,
    )

    def run(in_maps):
        ins = []
        for i, name in enumerate(in_names):
            if name == dbg_name:
                ins.append(dbg_zero)
            elif name in _REPLICATED:
                ins.append(in_maps[0][name])
            else:
                ins.append(np.concatenate(
                    [np.asarray(in_maps[c][name]) for c in range(NCORES)], axis=0))
        zeros = [np.zeros((NCORES * z.shape[0], *z.shape[1:]), z.dtype)
                 for z in zero_outs]
        out_arrs = sharded(*ins, *zeros)
        out_np = [np.asarray(a).reshape(NCORES, *out_avals[i].shape)
                  for i, a in enumerate(out_arrs)]
        return [
            {name: out_np[i][c] for i, name in enumerate(out_names)}
            for c in range(NCORES)
        ]

    return run


def get_runner(deg=DEG):
    if deg not in _RUNNER_CACHE:
        _RUNNER_CACHE[deg] = _make_runner(deg)
    return _RUNNER_CACHE[deg]


def kernel(z_mp, z_sc, W1, b1, W2, b2, pos):
    in_maps, deg = make_in_maps(z_mp, z_sc, W1, b1, W2, b2, pos)
    results = get_runner(deg)(in_maps)
    return combine_outputs(results, deg)
